# revision 12
# baseline (speedup 1.0000x reference)
"""Distributed Trainium2 Bass kernel for masked multi-head attention.

Problem: out = (softmax(scale * x Wq^T (x Wk^T)^T + mask * -1e5) (x Wv^T)) Wp^T + bp
  x [4, 2048, 768], mask [4, 2048, 2048], H=12 heads, D=64.

Sharding (8 cores): core = (batch b, head-group hg) with b = core//2,
hg = core%2 (6 heads each).  Column-parallel Wq/Wkv, row-parallel Wp;
each core produces a partial [2048, 768] output; the host sums the two
head-group partials per batch and adds the bias (the row-parallel
reduction), then stacks batches.

Device pipeline per core (softmax kept in the "S^T" layout [k, q] so the
attention-weight matrix is directly usable as matmul lhsT for P@V):
  phase 1: QT = WqT_scaled^T x^T, KT likewise, V = x Wv^T (fp32r matmuls)
  phase 2: per (head, k-tile pair): S^T tile = K^T slice x Q^T (PSUM ring),
    P = exp(S^T) on ScalarE (bf16), P *= (1-mask)^T on VectorE (bf16),
    O += P^T slices @ [V | 1] on PE (softmax denominator = 65th column).
  epilogue per q-chunk: O /= Z (reciprocal + broadcast multiply),
    transpose O on PE, out-project with row-sliced Wp (fp32r), DMA out.
"""

import os
from contextlib import ExitStack

import ml_dtypes
import numpy as np

import concourse.bass as bass
import concourse.tile as tile
from concourse import bacc, mybir
from concourse.bass_utils import run_bass_kernel_spmd
from concourse.masks import make_identity

B, N, C, H, D = 4, 2048, 768, 12, 64
SCALE = D ** -0.5
NCORES = 8
HGROUPS = 2
HL = H // HGROUPS          # 6 heads per group
CH = HL * D                # 384 channels per group
P = 128
NKT = N // P               # 16 k tiles
QCHUNK = 512
NQC = N // QCHUNK          # 4 q chunks
QSUBS = QCHUNK // P        # 4
CIN_T = C // P             # 6 input-channel tiles
CH_T = CH // P             # 3 group-channel tiles
E = D + 1                  # head slot width in O psum (64 V cols + 1 ones col)

F32 = mybir.dt.float32
F32R = mybir.dt.float32r
BF16 = mybir.dt.bfloat16


def build_kernel(debug=False):
    nc = bacc.Bacc("TRN2", target_bir_lowering=False, debug=False,
                   num_devices=NCORES)

    xT = nc.dram_tensor("xT", [C, N], BF16, kind="ExternalInput").ap()
    wqt = nc.dram_tensor("wqt", [C, CH], BF16, kind="ExternalInput").ap()
    wkt = nc.dram_tensor("wkt", [C, CH], BF16, kind="ExternalInput").ap()
    wvt = nc.dram_tensor("wvt", [C, CH], BF16, kind="ExternalInput").ap()
    wpt = nc.dram_tensor("wpt", [CH, C], BF16, kind="ExternalInput").ap()
    negmt = nc.dram_tensor("negmt", [N, N], BF16, kind="ExternalInput").ap()
    out = nc.dram_tensor("out", [N, C], F32, kind="ExternalOutput").ap()
    if debug:
        d_qt = nc.dram_tensor("d_qt", [P, N], BF16, kind="ExternalOutput").ap()
        d_kt = nc.dram_tensor("d_kt", [P, N], BF16, kind="ExternalOutput").ap()
        d_vp = nc.dram_tensor("d_vp", [P, HL, E], BF16, kind="ExternalOutput").ap()
        d_pm = nc.dram_tensor("d_pm", [P, 2, QCHUNK], BF16, kind="ExternalOutput").ap()
        d_osb = nc.dram_tensor("d_osb", [P, HL, D], F32, kind="ExternalOutput").ap()
        d_o = nc.dram_tensor("d_o", [P, HL * E], F32, kind="ExternalOutput").ap()
        d_otsb = nc.dram_tensor("d_otsb", [P, CH_T, QCHUNK], BF16, kind="ExternalOutput").ap()

    with tile.TileContext(nc) as tc, ExitStack() as ctx:
        persist = ctx.enter_context(tc.tile_pool(name="persist", bufs=1))
        # PSUM pools shared by both phases: "ring" slots are 2 banks each,
        # "ot" slots 1 bank each -> 2*2 + 4*1 = 8 banks total.
        ring_pool = ctx.enter_context(
            tc.tile_pool(name="ring", bufs=2, space="PSUM"))
        o_pool = ctx.enter_context(
            tc.tile_pool(name="opsum", bufs=4, space="PSUM"))

        qt_sb = [persist.tile([P, N], BF16, tag=f"qt{i}", name=f"qt{i}") for i in range(HL)]
        kt_sb = [persist.tile([P, N], BF16, tag=f"kt{i}", name=f"kt{i}") for i in range(HL)]
        vp_sb = [persist.tile([P, HL, E], BF16, tag=f"vp{j}", name=f"vp{j}")
                 for j in range(NKT)]
        wp_sb = [persist.tile([P, C], BF16, tag=f"wp{t}", name=f"wp{t}") for t in range(CH_T)]
        idn = persist.tile([P, P], BF16, tag="idn")

        # ---- phase 1: projections (emitted to overlap with the start of
        # attention: V first, then only heads 0/1's QT/KT; heads 2..5 are
        # produced mid-stream during q-chunk 0) ----
        ph1 = ctx.enter_context(tc.tile_pool(name="ph1", bufs=1))
        xt_sb = [ph1.tile([P, N], BF16, tag=f"xt{i}", name=f"xt{i}") for i in range(CIN_T)]
        wq_sb = [ph1.tile([P, CH], BF16, tag=f"wq{i}", name=f"wq{i}") for i in range(CIN_T)]
        wk_sb = [ph1.tile([P, CH], BF16, tag=f"wk{i}", name=f"wk{i}") for i in range(CIN_T)]
        wv_sb = [ph1.tile([P, CH], BF16, tag=f"wv{i}", name=f"wv{i}") for i in range(CIN_T)]
        for i in range(CIN_T):
            sl = slice(i * P, (i + 1) * P)
            nc.sync.dma_start(out=xt_sb[i], in_=xT[sl, :])
            nc.sync.dma_start(out=wv_sb[i], in_=wvt[sl, :])
        for i in range(CIN_T):
            sl = slice(i * P, (i + 1) * P)
            nc.sync.dma_start(out=wq_sb[i], in_=wqt[sl, :])
            nc.sync.dma_start(out=wk_sb[i], in_=wkt[sl, :])
        for t in range(CH_T):
            nc.sync.dma_start(out=wp_sb[t], in_=wpt[t * P:(t + 1) * P, :])

        # Zero the pad rows (64..127) of per-head QT/KT so the QK stationary
        # operand is a uniform [128, 128] tile; heads 0/1 first so the
        # attention stream can start as soon as possible.
        for t in (0, 1):
            nc.gpsimd.memset(qt_sb[t][D:P, :], 0.0)
            nc.gpsimd.memset(kt_sb[t][D:P, :], 0.0)

        # V: [N, CH] = x @ Wv^T into the ones-augmented bf16 layout
        # vp[j] = [P, HL, 65] with vp[..., 64] == 1.0.  Emitted before any
        # otiles so its "ot"-tag psum slots are scheduled first.
        for j in range(NKT):
            ps = o_pool.tile([P, CH], F32, tag="ot")
            for ci in range(CIN_T):
                nc.tensor.matmul(
                    ps,
                    xt_sb[ci][:, j * P:(j + 1) * P],
                    wv_sb[ci],
                    start=(ci == 0), stop=(ci == CIN_T - 1))
            nc.gpsimd.memset(vp_sb[j], 1.0)
            nc.vector.tensor_copy(
                vp_sb[j][:, :, 0:D],
                ps.rearrange("p (h d) -> p h d", h=HL))

        def emit_qtkt_chunk(m, which, nck):
            dst, w_sb, evict = (
                (qt_sb, wq_sb, "act") if which == 0 else (kt_sb, wk_sb, "dve"))
            if True:
                if True:
                    ps = ring_pool.tile([P, 512], F32, tag="ring",
                                        name=f"p1ps{m}_{which}_{nck}")
                    for ci in range(CIN_T):
                        nc.tensor.matmul(
                            ps,
                            w_sb[ci][:, m * P:(m + 1) * P],
                            xt_sb[ci][:, nck * 512:(nck + 1) * 512],
                            start=(ci == 0), stop=(ci == CIN_T - 1))
                    for sub in range(2):
                        dtile = dst[2 * m + sub]
                        dslice = dtile[0:D, nck * 512:(nck + 1) * 512]
                        pslice = ps[sub * D:(sub + 1) * D, :]
                        if evict == "act":
                            nc.scalar.copy(dslice, pslice)
                        else:
                            nc.vector.tensor_copy(dslice, pslice)

        def emit_qtkt(m):
            for which in range(2):
                for nck in range(N // 512):
                    emit_qtkt_chunk(m, which, nck)

        emit_qtkt(0)
        for t in range(2, HL):
            nc.gpsimd.memset(qt_sb[t][D:P, :], 0.0)
            nc.gpsimd.memset(kt_sb[t][D:P, :], 0.0)
        make_identity(nc, idn)

        if debug:
            nc.sync.dma_start(out=d_qt, in_=qt_sb[0])
            nc.sync.dma_start(out=d_kt, in_=kt_sb[0])
            nc.sync.dma_start(out=d_vp, in_=vp_sb[0])

        # ---- phase 2: attention ----
        mpool = ctx.enter_context(tc.tile_pool(name="mask", bufs=2))
        p_pool = ctx.enter_context(tc.tile_pool(name="pexp", bufs=6))
        pm_pool = ctx.enter_context(tc.tile_pool(name="pmask", bufs=6))
        epi = ctx.enter_context(tc.tile_pool(name="epi", bufs=8))
        ot_pool = ctx.enter_context(tc.tile_pool(name="otsb", bufs=2))
        outsb_pool = ctx.enter_context(tc.tile_pool(name="outsb", bufs=4))

        mchunks = ([(1, w, n) for w in range(2) for n in range(N // 512)] +
                   [(2, w, n) for w in range(2) for n in range(N // 512)])

        def make_epilogue(qc, q0, otiles):
            def epi_fn():
                # all divisions first (frees O psum slots), then transposes +
                # projection draw psum from the freed "ot" slots so the next
                # q-chunk's QK/exp stream keeps the ring pool.
                otsb = ot_pool.tile([P, CH_T, QCHUNK], BF16, tag="otsb",
                                    name=f"otsb{qc}")
                osbs = []
                for s in range(QSUBS):
                    otv = otiles[s].rearrange("p (h e) -> p h e", h=HL)
                    zrec = epi.tile([P, HL], F32, tag="zr", name=f"zr{qc}_{s}")
                    nc.vector.reciprocal(zrec, otv[:, :, D])
                    osb = epi.tile([P, HL, D], BF16, tag="osb",
                                   name=f"osb{qc}_{s}")
                    zb = bass.AP(
                        tensor=zrec.tensor, offset=zrec.offset,
                        ap=[*zrec.ap, [0, D]])
                    nc.vector.tensor_mul(osb, otv[:, :, 0:D], zb)
                    osbs.append(osb)
                for s in range(QSUBS):
                    osf = osbs[s].rearrange("p h d -> p (h d)")
                    otp = o_pool.tile([P, CH_T, P], BF16, tag="ot",
                                      name=f"otp{qc}_{s}")
                    for ct in range(CH_T):
                        nc.tensor.transpose(
                            otp[:, ct, :], osf[:, ct * P:(ct + 1) * P], idn)
                    nc.vector.tensor_copy(otsb[:, :, s * P:(s + 1) * P], otp)
                    if debug and qc == 0 and s == 0:
                        nc.sync.dma_start(out=d_osb, in_=osbs[s])
                if debug and qc == 0:
                    nc.sync.dma_start(out=d_otsb, in_=otsb)
                for s in range(QSUBS):
                    ppss = []
                    for cf, (c0, c1) in enumerate(((0, CH), (CH, C))):
                        pps = o_pool.tile([P, CH], F32, tag="ot",
                                          name=f"pps{qc}_{s}_{cf}")
                        for ct in range(CH_T):
                            nc.tensor.matmul(
                                pps,
                                otsb[:, ct, s * P:(s + 1) * P],
                                wp_sb[ct][:, c0:c1],
                                start=(ct == 0), stop=(ct == CH_T - 1))
                        ppss.append(pps)
                    ob = outsb_pool.tile([P, C], F32, tag="ob",
                                         name=f"ob{qc}_{s}")
                    nc.vector.tensor_copy(ob[:, 0:CH], ppss[0])
                    nc.vector.tensor_copy(ob[:, CH:C], ppss[1])
                    nc.sync.dma_start(
                        out=out[q0 + s * P:q0 + (s + 1) * P, :], in_=ob)
            return epi_fn

        pending_epi = None
        for qc in range(NQC):
            q0 = qc * QCHUNK
            mk = mpool.tile([P, NKT, QCHUNK], BF16, tag="mk")
            for j in range(NKT):
                nc.sync.dma_start(
                    out=mk[:, j, :],
                    in_=negmt[j * P:(j + 1) * P, q0:q0 + QCHUNK])

            otiles = [o_pool.tile([P, HL * E], F32, tag="ot",
                                  name=f"otile{qc}_{s_}")
                      for s_ in range(QSUBS)]

            for h in range(HL):
                kth = kt_sb[h]
                qth = qt_sb[h]
                for ktp in range(NKT // 2):
                    ring = ring_pool.tile([P, 2, QCHUNK], F32, tag="ring")
                    for u in range(2):
                        kti = 2 * ktp + u
                        nc.tensor.matmul(
                            ring[:, u, :],
                            kth[:, kti * P:(kti + 1) * P],
                            qth[:, q0:q0 + QCHUNK],
                            start=True, stop=True)
                    pexp = p_pool.tile([P, 2, QCHUNK], BF16, tag="pe")
                    nc.scalar.activation(
                        pexp, ring, mybir.ActivationFunctionType.Exp)
                    pm = pm_pool.tile([P, 2, QCHUNK], BF16, tag="pm")
                    nc.vector.tensor_mul(
                        pm, pexp, mk[:, 2 * ktp:2 * ktp + 2, :])
                    if debug and qc == 0 and h == 0 and ktp == 0:
                        nc.sync.dma_start(out=d_pm, in_=pm)
                    for u in range(2):
                        kti = 2 * ktp + u
                        for s in range(QSUBS):
                            nc.tensor.matmul(
                                otiles[s][:, h * E:(h + 1) * E],
                                pm[:, u, s * P:(s + 1) * P],
                                vp_sb[kti][:, h, :],
                                start=(kti == 0), stop=(kti == NKT - 1))
                    # spread the remaining QT/KT build (heads 2..5) one
                    # chunk at a time through q-chunk 0's stream
                    if qc == 0 and h >= 1 and mchunks and (ktp % 3) == 2:
                        emit_qtkt_chunk(*mchunks.pop(0))
                if qc == 0 and h == 1 and pending_epi is None:
                    # ensure head 2/3 data exists before it is needed
                    while len(mchunks) > 8:
                        emit_qtkt_chunk(*mchunks.pop(0))
                if qc == 0 and h == 3:
                    while mchunks:
                        emit_qtkt_chunk(*mchunks.pop(0))
                if h == 0 and pending_epi is not None:
                    pending_epi()
                    pending_epi = None
            pending_epi = make_epilogue(qc, q0, otiles)
        pending_epi()

    nc.compile()
    return nc


_CACHE = {}


def _get_nc():
    debug = os.environ.get("KERNEL_DEBUG", "0") == "1"
    key = ("nc", debug)
    if key not in _CACHE:
        _CACHE[key] = build_kernel(debug=debug)
    return _CACHE[key]


def kernel(x, mask, Wq, Wkv, Wp, bp):
    x = np.asarray(x, np.float32)
    mask = np.asarray(mask, np.float32)
    Wq = np.asarray(Wq, np.float32)
    Wkv = np.asarray(Wkv, np.float32)
    Wp = np.asarray(Wp, np.float32)
    bp = np.asarray(bp, np.float32)

    nc = _get_nc()
    in_maps = []
    for core in range(NCORES):
        b, hg = divmod(core, HGROUPS)
        rows = slice(hg * CH, (hg + 1) * CH)
        in_maps.append({
            "xT": np.ascontiguousarray(x[b].T.astype(ml_dtypes.bfloat16)),
            "wqt": np.ascontiguousarray(((Wq[rows, :] * SCALE).T).astype(ml_dtypes.bfloat16)),
            "wkt": np.ascontiguousarray(Wkv[rows, :].T.astype(ml_dtypes.bfloat16)),
            "wvt": np.ascontiguousarray(Wkv.T[:, C + hg * CH:C + (hg + 1) * CH].astype(ml_dtypes.bfloat16)),
            "wpt": np.ascontiguousarray(Wp[:, rows].T.astype(ml_dtypes.bfloat16)),
            "negmt": np.ascontiguousarray(
                (1.0 - mask[b].T).astype(ml_dtypes.bfloat16)),
        })

    trace = os.environ.get("KERNEL_TRACE", "0") == "1"
    res = run_bass_kernel_spmd(nc, in_maps, core_ids=list(range(NCORES)),
                               trace=trace)
    kernel.last_results = res

    outs = [res.results[i]["out"] for i in range(NCORES)]
    full = np.empty((B, N, C), np.float32)
    for b in range(B):
        full[b] = outs[2 * b] + outs[2 * b + 1] + bp[None, :]
    return full


# revision 13
# speedup vs baseline: 1.0099x; 1.0099x over previous
"""Distributed Trainium2 Bass kernel for masked multi-head attention.

Problem: out = (softmax(scale * x Wq^T (x Wk^T)^T + mask * -1e5) (x Wv^T)) Wp^T + bp
  x [4, 2048, 768], mask [4, 2048, 2048], H=12 heads, D=64.

Sharding (8 cores): core = (batch b, head-group hg) with b = core//2,
hg = core%2 (6 heads each).  Column-parallel Wq/Wkv, row-parallel Wp;
each core produces a partial [2048, 768] output; the host sums the two
head-group partials per batch and adds the bias (the row-parallel
reduction), then stacks batches.

Device pipeline per core (softmax kept in the "S^T" layout [k, q] so the
attention-weight matrix is directly usable as matmul lhsT for P@V):
  phase 1: QT = WqT_scaled^T x^T, KT likewise, V = x Wv^T (fp32r matmuls)
  phase 2: per (head, k-tile pair): S^T tile = K^T slice x Q^T (PSUM ring),
    P = exp(S^T) on ScalarE (bf16), P *= (1-mask)^T on VectorE (bf16),
    O += P^T slices @ [V | 1] on PE (softmax denominator = 65th column).
  epilogue per q-chunk: O /= Z (reciprocal + broadcast multiply),
    transpose O on PE, out-project with row-sliced Wp (fp32r), DMA out.
"""

import os
from contextlib import ExitStack

import ml_dtypes
import numpy as np

import concourse.bass as bass
import concourse.tile as tile
from concourse import bacc, mybir
from concourse.bass_utils import run_bass_kernel_spmd
from concourse.masks import make_identity

B, N, C, H, D = 4, 2048, 768, 12, 64
SCALE = D ** -0.5
NCORES = 8
HGROUPS = 2
HL = H // HGROUPS          # 6 heads per group
CH = HL * D                # 384 channels per group
P = 128
NKT = N // P               # 16 k tiles
QCHUNK = 512
NQC = N // QCHUNK          # 4 q chunks
QSUBS = QCHUNK // P        # 4
CIN_T = C // P             # 6 input-channel tiles
CH_T = CH // P             # 3 group-channel tiles
E = D + 1                  # head slot width in O psum (64 V cols + 1 ones col)

F32 = mybir.dt.float32
F32R = mybir.dt.float32r
BF16 = mybir.dt.bfloat16


def build_kernel(debug=False):
    nc = bacc.Bacc("TRN2", target_bir_lowering=False, debug=False,
                   num_devices=NCORES)

    xT = nc.dram_tensor("xT", [C, N], BF16, kind="ExternalInput").ap()
    wqt = nc.dram_tensor("wqt", [C, CH], BF16, kind="ExternalInput").ap()
    wkt = nc.dram_tensor("wkt", [C, CH], BF16, kind="ExternalInput").ap()
    wvt = nc.dram_tensor("wvt", [C, CH], BF16, kind="ExternalInput").ap()
    wpt = nc.dram_tensor("wpt", [CH, C], BF16, kind="ExternalInput").ap()
    negmt = nc.dram_tensor("negmt", [N, N], BF16, kind="ExternalInput").ap()
    out = nc.dram_tensor("out", [N, C], F32, kind="ExternalOutput").ap()
    if debug:
        d_qt = nc.dram_tensor("d_qt", [P, N], BF16, kind="ExternalOutput").ap()
        d_kt = nc.dram_tensor("d_kt", [P, N], BF16, kind="ExternalOutput").ap()
        d_vp = nc.dram_tensor("d_vp", [P, HL, E], BF16, kind="ExternalOutput").ap()
        d_pm = nc.dram_tensor("d_pm", [P, 2, QCHUNK], BF16, kind="ExternalOutput").ap()
        d_osb = nc.dram_tensor("d_osb", [P, HL, D], F32, kind="ExternalOutput").ap()
        d_o = nc.dram_tensor("d_o", [P, HL * E], F32, kind="ExternalOutput").ap()
        d_otsb = nc.dram_tensor("d_otsb", [P, CH_T, QCHUNK], BF16, kind="ExternalOutput").ap()

    with tile.TileContext(nc) as tc, ExitStack() as ctx:
        persist = ctx.enter_context(tc.tile_pool(name="persist", bufs=1))
        # PSUM pools shared by both phases: "ring" slots are 2 banks each,
        # "ot" slots 1 bank each -> 2*2 + 4*1 = 8 banks total.
        ring_pool = ctx.enter_context(
            tc.tile_pool(name="ring", bufs=2, space="PSUM"))
        o_pool = ctx.enter_context(
            tc.tile_pool(name="opsum", bufs=4, space="PSUM"))

        qt_sb = [persist.tile([P, N], BF16, tag=f"qt{i}", name=f"qt{i}") for i in range(HL)]
        kt_sb = [persist.tile([P, N], BF16, tag=f"kt{i}", name=f"kt{i}") for i in range(HL)]
        vp_sb = [persist.tile([P, HL, E], BF16, tag=f"vp{j}", name=f"vp{j}")
                 for j in range(NKT)]
        wp_sb = [persist.tile([P, C], BF16, tag=f"wp{t}", name=f"wp{t}") for t in range(CH_T)]
        idn = persist.tile([P, P], BF16, tag="idn")

        # ---- phase 1: projections (emitted to overlap with the start of
        # attention: V first, then only heads 0/1's QT/KT; heads 2..5 are
        # produced mid-stream during q-chunk 0) ----
        ph1 = ctx.enter_context(tc.tile_pool(name="ph1", bufs=1))
        xt_sb = [ph1.tile([P, N], BF16, tag=f"xt{i}", name=f"xt{i}") for i in range(CIN_T)]
        wq_sb = [ph1.tile([P, CH], BF16, tag=f"wq{i}", name=f"wq{i}") for i in range(CIN_T)]
        wk_sb = [ph1.tile([P, CH], BF16, tag=f"wk{i}", name=f"wk{i}") for i in range(CIN_T)]
        wv_sb = [ph1.tile([P, CH], BF16, tag=f"wv{i}", name=f"wv{i}") for i in range(CIN_T)]
        for i in range(CIN_T):
            sl = slice(i * P, (i + 1) * P)
            nc.sync.dma_start(out=xt_sb[i], in_=xT[sl, :])
            nc.sync.dma_start(out=wv_sb[i], in_=wvt[sl, :])
        for i in range(CIN_T):
            sl = slice(i * P, (i + 1) * P)
            nc.sync.dma_start(out=wq_sb[i], in_=wqt[sl, :])
            nc.sync.dma_start(out=wk_sb[i], in_=wkt[sl, :])
        for t in range(CH_T):
            nc.sync.dma_start(out=wp_sb[t], in_=wpt[t * P:(t + 1) * P, :])

        # Zero the pad rows (64..127) of per-head QT/KT so the QK stationary
        # operand is a uniform [128, 128] tile; heads 0/1 first so the
        # attention stream can start as soon as possible.
        for t in (0, 1):
            nc.gpsimd.memset(qt_sb[t][D:P, :], 0.0)
            nc.gpsimd.memset(kt_sb[t][D:P, :], 0.0)

        # V: [N, CH] = x @ Wv^T into the ones-augmented bf16 layout
        # vp[j] = [P, HL, 65] with vp[..., 64] == 1.0.  Emitted before any
        # otiles so its "ot"-tag psum slots are scheduled first.
        for j in range(NKT):
            ps = o_pool.tile([P, CH], F32, tag="ot")
            for ci in range(CIN_T):
                nc.tensor.matmul(
                    ps,
                    xt_sb[ci][:, j * P:(j + 1) * P],
                    wv_sb[ci],
                    start=(ci == 0), stop=(ci == CIN_T - 1))
            nc.gpsimd.memset(vp_sb[j], 1.0)
            nc.vector.tensor_copy(
                vp_sb[j][:, :, 0:D],
                ps.rearrange("p (h d) -> p h d", h=HL))

        def emit_qtkt_chunk(m, which, nck):
            dst, w_sb, evict = (
                (qt_sb, wq_sb, "act") if which == 0 else (kt_sb, wk_sb, "dve"))
            if True:
                if True:
                    ps = ring_pool.tile([P, 512], F32, tag="ring",
                                        name=f"p1ps{m}_{which}_{nck}")
                    for ci in range(CIN_T):
                        nc.tensor.matmul(
                            ps,
                            w_sb[ci][:, m * P:(m + 1) * P],
                            xt_sb[ci][:, nck * 512:(nck + 1) * 512],
                            start=(ci == 0), stop=(ci == CIN_T - 1))
                    for sub in range(2):
                        dtile = dst[2 * m + sub]
                        dslice = dtile[0:D, nck * 512:(nck + 1) * 512]
                        pslice = ps[sub * D:(sub + 1) * D, :]
                        if evict == "act":
                            nc.scalar.copy(dslice, pslice)
                        else:
                            nc.vector.tensor_copy(dslice, pslice)

        def emit_qtkt(m):
            for which in range(2):
                for nck in range(N // 512):
                    emit_qtkt_chunk(m, which, nck)

        for t in range(2, HL):
            nc.gpsimd.memset(qt_sb[t][D:P, :], 0.0)
            nc.gpsimd.memset(kt_sb[t][D:P, :], 0.0)
        emit_qtkt(0)
        emit_qtkt(1)
        emit_qtkt(2)
        make_identity(nc, idn)

        if debug:
            nc.sync.dma_start(out=d_qt, in_=qt_sb[0])
            nc.sync.dma_start(out=d_kt, in_=kt_sb[0])
            nc.sync.dma_start(out=d_vp, in_=vp_sb[0])

        # ---- phase 2: attention ----
        mpool = ctx.enter_context(tc.tile_pool(name="mask", bufs=2))
        p_pool = ctx.enter_context(tc.tile_pool(name="pexp", bufs=6))
        pm_pool = ctx.enter_context(tc.tile_pool(name="pmask", bufs=6))
        epi = ctx.enter_context(tc.tile_pool(name="epi", bufs=8))
        ot_pool = ctx.enter_context(tc.tile_pool(name="otsb", bufs=2))
        outsb_pool = ctx.enter_context(tc.tile_pool(name="outsb", bufs=4))

        def make_epilogue(qc, q0, otiles):
            def epi_fn():
                # all divisions first (frees O psum slots), then transposes +
                # projection draw psum from the freed "ot" slots so the next
                # q-chunk's QK/exp stream keeps the ring pool.
                otsb = ot_pool.tile([P, CH_T, QCHUNK], BF16, tag="otsb",
                                    name=f"otsb{qc}")
                osbs = []
                for s in range(QSUBS):
                    otv = otiles[s].rearrange("p (h e) -> p h e", h=HL)
                    zrec = epi.tile([P, HL], F32, tag="zr", name=f"zr{qc}_{s}")
                    nc.vector.reciprocal(zrec, otv[:, :, D])
                    osb = epi.tile([P, HL, D], BF16, tag="osb",
                                   name=f"osb{qc}_{s}")
                    zb = bass.AP(
                        tensor=zrec.tensor, offset=zrec.offset,
                        ap=[*zrec.ap, [0, D]])
                    nc.vector.tensor_mul(osb, otv[:, :, 0:D], zb)
                    osbs.append(osb)
                for s in range(QSUBS):
                    osf = osbs[s].rearrange("p h d -> p (h d)")
                    otp = o_pool.tile([P, CH_T, P], BF16, tag="ot",
                                      name=f"otp{qc}_{s}")
                    for ct in range(CH_T):
                        nc.tensor.transpose(
                            otp[:, ct, :], osf[:, ct * P:(ct + 1) * P], idn)
                    nc.vector.tensor_copy(otsb[:, :, s * P:(s + 1) * P], otp)
                    if debug and qc == 0 and s == 0:
                        nc.sync.dma_start(out=d_osb, in_=osbs[s])
                if debug and qc == 0:
                    nc.sync.dma_start(out=d_otsb, in_=otsb)
                for s in range(QSUBS):
                    ppss = []
                    for cf, (c0, c1) in enumerate(((0, CH), (CH, C))):
                        pps = o_pool.tile([P, CH], F32, tag="ot",
                                          name=f"pps{qc}_{s}_{cf}")
                        for ct in range(CH_T):
                            nc.tensor.matmul(
                                pps,
                                otsb[:, ct, s * P:(s + 1) * P],
                                wp_sb[ct][:, c0:c1],
                                start=(ct == 0), stop=(ct == CH_T - 1))
                        ppss.append(pps)
                    ob = outsb_pool.tile([P, C], F32, tag="ob",
                                         name=f"ob{qc}_{s}")
                    nc.vector.tensor_copy(ob[:, 0:CH], ppss[0])
                    nc.vector.tensor_copy(ob[:, CH:C], ppss[1])
                    nc.sync.dma_start(
                        out=out[q0 + s * P:q0 + (s + 1) * P, :], in_=ob)
            return epi_fn

        pending_epi = None
        for qc in range(NQC):
            q0 = qc * QCHUNK
            mk = mpool.tile([P, NKT, QCHUNK], BF16, tag="mk")
            for j in range(NKT):
                nc.sync.dma_start(
                    out=mk[:, j, :],
                    in_=negmt[j * P:(j + 1) * P, q0:q0 + QCHUNK])

            otiles = [o_pool.tile([P, HL * E], F32, tag="ot",
                                  name=f"otile{qc}_{s_}")
                      for s_ in range(QSUBS)]

            for h in range(HL):
                kth = kt_sb[h]
                qth = qt_sb[h]
                for ktp in range(NKT // 2):
                    ring = ring_pool.tile([P, 2, QCHUNK], F32, tag="ring")
                    for u in range(2):
                        kti = 2 * ktp + u
                        nc.tensor.matmul(
                            ring[:, u, :],
                            kth[:, kti * P:(kti + 1) * P],
                            qth[:, q0:q0 + QCHUNK],
                            start=True, stop=True)
                    pexp = p_pool.tile([P, 2, QCHUNK], BF16, tag="pe")
                    nc.scalar.activation(
                        pexp, ring, mybir.ActivationFunctionType.Exp)
                    pm = pm_pool.tile([P, 2, QCHUNK], BF16, tag="pm")
                    nc.vector.tensor_mul(
                        pm, pexp, mk[:, 2 * ktp:2 * ktp + 2, :])
                    if debug and qc == 0 and h == 0 and ktp == 0:
                        nc.sync.dma_start(out=d_pm, in_=pm)
                    for u in range(2):
                        kti = 2 * ktp + u
                        for s in range(QSUBS):
                            nc.tensor.matmul(
                                otiles[s][:, h * E:(h + 1) * E],
                                pm[:, u, s * P:(s + 1) * P],
                                vp_sb[kti][:, h, :],
                                start=(kti == 0), stop=(kti == NKT - 1))
                if h == 0 and pending_epi is not None:
                    pending_epi()
                    pending_epi = None
            pending_epi = make_epilogue(qc, q0, otiles)
        pending_epi()

    nc.compile()
    return nc


_CACHE = {}


def _get_nc():
    debug = os.environ.get("KERNEL_DEBUG", "0") == "1"
    key = ("nc", debug)
    if key not in _CACHE:
        _CACHE[key] = build_kernel(debug=debug)
    return _CACHE[key]


def kernel(x, mask, Wq, Wkv, Wp, bp):
    x = np.asarray(x, np.float32)
    mask = np.asarray(mask, np.float32)
    Wq = np.asarray(Wq, np.float32)
    Wkv = np.asarray(Wkv, np.float32)
    Wp = np.asarray(Wp, np.float32)
    bp = np.asarray(bp, np.float32)

    nc = _get_nc()
    in_maps = []
    for core in range(NCORES):
        b, hg = divmod(core, HGROUPS)
        rows = slice(hg * CH, (hg + 1) * CH)
        in_maps.append({
            "xT": np.ascontiguousarray(x[b].T.astype(ml_dtypes.bfloat16)),
            "wqt": np.ascontiguousarray(((Wq[rows, :] * SCALE).T).astype(ml_dtypes.bfloat16)),
            "wkt": np.ascontiguousarray(Wkv[rows, :].T.astype(ml_dtypes.bfloat16)),
            "wvt": np.ascontiguousarray(Wkv.T[:, C + hg * CH:C + (hg + 1) * CH].astype(ml_dtypes.bfloat16)),
            "wpt": np.ascontiguousarray(Wp[:, rows].T.astype(ml_dtypes.bfloat16)),
            "negmt": np.ascontiguousarray(
                (1.0 - mask[b].T).astype(ml_dtypes.bfloat16)),
        })

    trace = os.environ.get("KERNEL_TRACE", "0") == "1"
    res = run_bass_kernel_spmd(nc, in_maps, core_ids=list(range(NCORES)),
                               trace=trace)
    kernel.last_results = res

    outs = [res.results[i]["out"] for i in range(NCORES)]
    full = np.empty((B, N, C), np.float32)
    for b in range(B):
        full[b] = outs[2 * b] + outs[2 * b + 1] + bp[None, :]
    return full


# revision 14
# speedup vs baseline: 1.0200x; 1.0101x over previous
"""Distributed Trainium2 Bass kernel for masked multi-head attention.

Problem: out = (softmax(scale * x Wq^T (x Wk^T)^T + mask * -1e5) (x Wv^T)) Wp^T + bp
  x [4, 2048, 768], mask [4, 2048, 2048], H=12 heads, D=64.

Sharding (8 cores): core = (batch b, head-group hg) with b = core//2,
hg = core%2 (6 heads each).  Column-parallel Wq/Wkv, row-parallel Wp;
each core produces a partial [2048, 768] output; the host sums the two
head-group partials per batch and adds the bias (the row-parallel
reduction), then stacks batches.

Device pipeline per core (softmax kept in the "S^T" layout [k, q] so the
attention-weight matrix is directly usable as matmul lhsT for P@V):
  phase 1: QT = WqT_scaled^T x^T, KT likewise, V = x Wv^T (fp32r matmuls)
  phase 2: per (head, k-tile pair): S^T tile = K^T slice x Q^T (PSUM ring),
    P = exp(S^T) on ScalarE (bf16), P *= (1-mask)^T on VectorE (bf16),
    O += P^T slices @ [V | 1] on PE (softmax denominator = 65th column).
  epilogue per q-chunk: O /= Z (reciprocal + broadcast multiply),
    transpose O on PE, out-project with row-sliced Wp (fp32r), DMA out.
"""

import os
from contextlib import ExitStack

import ml_dtypes
import numpy as np

import sys
import types

try:  # defensive: concourse's trace path imports this; absent on some images
    import antenv.axon_hooks  # noqa: F401
except ImportError:
    try:
        import antenv
        _m = types.ModuleType('antenv.axon_hooks')
        _m._hook = None
        _m.set_axon_ntff_profile_hook = lambda h: setattr(_m, '_hook', h)
        _m.get_axon_ntff_profile_hook = lambda: _m._hook
        sys.modules['antenv.axon_hooks'] = _m
        antenv.axon_hooks = _m
    except ImportError:
        pass

import concourse.bass as bass
import concourse.tile as tile
from concourse import bacc, mybir
from concourse.bass_utils import run_bass_kernel_spmd
from concourse.masks import make_identity

B, N, C, H, D = 4, 2048, 768, 12, 64
SCALE = D ** -0.5
NCORES = 8
HGROUPS = 2
HL = H // HGROUPS          # 6 heads per group
CH = HL * D                # 384 channels per group
P = 128
NKT = N // P               # 16 k tiles
QCHUNK = 512
NQC = N // QCHUNK          # 4 q chunks
QSUBS = QCHUNK // P        # 4
CIN_T = C // P             # 6 input-channel tiles
CH_T = CH // P             # 3 group-channel tiles
E = D + 1                  # head slot width in O psum (64 V cols + 1 ones col)

F32 = mybir.dt.float32
F32R = mybir.dt.float32r
BF16 = mybir.dt.bfloat16


def build_kernel(debug=False):
    nc = bacc.Bacc("TRN2", target_bir_lowering=False, debug=False,
                   num_devices=NCORES)

    xT = nc.dram_tensor("xT", [C, N], BF16, kind="ExternalInput").ap()
    wqt = nc.dram_tensor("wqt", [C, CH], BF16, kind="ExternalInput").ap()
    wkt = nc.dram_tensor("wkt", [C, CH], BF16, kind="ExternalInput").ap()
    wvt = nc.dram_tensor("wvt", [C, CH], BF16, kind="ExternalInput").ap()
    wpt = nc.dram_tensor("wpt", [CH, C], BF16, kind="ExternalInput").ap()
    negmt = nc.dram_tensor("negmt", [N, N], BF16, kind="ExternalInput").ap()
    out = nc.dram_tensor("out", [N, C], F32, kind="ExternalOutput").ap()
    if debug:
        d_qt = nc.dram_tensor("d_qt", [P, N], BF16, kind="ExternalOutput").ap()
        d_kt = nc.dram_tensor("d_kt", [P, N], BF16, kind="ExternalOutput").ap()
        d_vp = nc.dram_tensor("d_vp", [P, HL, E], BF16, kind="ExternalOutput").ap()
        d_pm = nc.dram_tensor("d_pm", [P, 2, QCHUNK], BF16, kind="ExternalOutput").ap()
        d_osb = nc.dram_tensor("d_osb", [P, HL, D], F32, kind="ExternalOutput").ap()
        d_o = nc.dram_tensor("d_o", [P, HL * E], F32, kind="ExternalOutput").ap()
        d_otsb = nc.dram_tensor("d_otsb", [P, CH_T, QCHUNK], BF16, kind="ExternalOutput").ap()

    with tile.TileContext(nc) as tc, ExitStack() as ctx:
        persist = ctx.enter_context(tc.tile_pool(name="persist", bufs=1))
        # PSUM pools shared by both phases: "ring" slots are 2 banks each,
        # "ot" slots 1 bank each -> 2*2 + 4*1 = 8 banks total.
        ring_pool = ctx.enter_context(
            tc.tile_pool(name="ring", bufs=2, space="PSUM"))
        o_pool = ctx.enter_context(
            tc.tile_pool(name="opsum", bufs=4, space="PSUM"))

        qt_sb = [persist.tile([P, N], BF16, tag=f"qt{i}", name=f"qt{i}") for i in range(HL)]
        kt_sb = [persist.tile([P, N], BF16, tag=f"kt{i}", name=f"kt{i}") for i in range(HL)]
        vp_sb = [persist.tile([P, HL, E], BF16, tag=f"vp{j}", name=f"vp{j}")
                 for j in range(NKT)]
        wp_sb = [persist.tile([P, C], BF16, tag=f"wp{t}", name=f"wp{t}") for t in range(CH_T)]
        idn = persist.tile([P, P], BF16, tag="idn")

        # ---- phase 1: projections (emitted to overlap with the start of
        # attention: V first, then only heads 0/1's QT/KT; heads 2..5 are
        # produced mid-stream during q-chunk 0) ----
        ph1 = ctx.enter_context(tc.tile_pool(name="ph1", bufs=1))
        xt_sb = [ph1.tile([P, N], BF16, tag=f"xt{i}", name=f"xt{i}") for i in range(CIN_T)]
        wq_sb = [ph1.tile([P, CH], BF16, tag=f"wq{i}", name=f"wq{i}") for i in range(CIN_T)]
        wk_sb = [ph1.tile([P, CH], BF16, tag=f"wk{i}", name=f"wk{i}") for i in range(CIN_T)]
        wv_sb = [ph1.tile([P, CH], BF16, tag=f"wv{i}", name=f"wv{i}") for i in range(CIN_T)]
        for i in range(CIN_T):
            sl = slice(i * P, (i + 1) * P)
            nc.sync.dma_start(out=xt_sb[i], in_=xT[sl, :])
            nc.sync.dma_start(out=wv_sb[i], in_=wvt[sl, :])
        for i in range(CIN_T):
            sl = slice(i * P, (i + 1) * P)
            nc.sync.dma_start(out=wq_sb[i], in_=wqt[sl, :])
            nc.sync.dma_start(out=wk_sb[i], in_=wkt[sl, :])
        for t in range(CH_T):
            nc.sync.dma_start(out=wp_sb[t], in_=wpt[t * P:(t + 1) * P, :])

        # Zero the pad rows (64..127) of per-head QT/KT so the QK stationary
        # operand is a uniform [128, 128] tile; heads 0/1 first so the
        # attention stream can start as soon as possible.
        for t in (0, 1):
            nc.gpsimd.memset(qt_sb[t][D:P, :], 0.0)
            nc.gpsimd.memset(kt_sb[t][D:P, :], 0.0)

        # V: [N, CH] = x @ Wv^T into the ones-augmented bf16 layout
        # vp[j] = [P, HL, 65] with vp[..., 64] == 1.0.  Emitted before any
        # otiles so its "ot"-tag psum slots are scheduled first.
        for j in range(NKT):
            ps = o_pool.tile([P, CH], F32, tag="ot")
            for ci in range(CIN_T):
                nc.tensor.matmul(
                    ps,
                    xt_sb[ci][:, j * P:(j + 1) * P],
                    wv_sb[ci],
                    start=(ci == 0), stop=(ci == CIN_T - 1))
            nc.gpsimd.memset(vp_sb[j], 1.0)
            nc.vector.tensor_copy(
                vp_sb[j][:, :, 0:D],
                ps.rearrange("p (h d) -> p h d", h=HL))

        def emit_qtkt_chunk(m, which, nck):
            dst, w_sb, evict = (
                (qt_sb, wq_sb, "act") if which == 0 else (kt_sb, wk_sb, "dve"))
            if True:
                if True:
                    ps = ring_pool.tile([P, 512], F32, tag="ring",
                                        name=f"p1ps{m}_{which}_{nck}")
                    for ci in range(CIN_T):
                        nc.tensor.matmul(
                            ps,
                            w_sb[ci][:, m * P:(m + 1) * P],
                            xt_sb[ci][:, nck * 512:(nck + 1) * 512],
                            start=(ci == 0), stop=(ci == CIN_T - 1))
                    for sub in range(2):
                        dtile = dst[2 * m + sub]
                        dslice = dtile[0:D, nck * 512:(nck + 1) * 512]
                        pslice = ps[sub * D:(sub + 1) * D, :]
                        if evict == "act":
                            nc.scalar.copy(dslice, pslice)
                        else:
                            nc.vector.tensor_copy(dslice, pslice)

        def emit_qtkt(m):
            for which in range(2):
                for nck in range(N // 512):
                    emit_qtkt_chunk(m, which, nck)

        for t in range(2, HL):
            nc.gpsimd.memset(qt_sb[t][D:P, :], 0.0)
            nc.gpsimd.memset(kt_sb[t][D:P, :], 0.0)
        emit_qtkt(0)
        emit_qtkt(1)
        emit_qtkt(2)
        make_identity(nc, idn)

        if debug:
            nc.sync.dma_start(out=d_qt, in_=qt_sb[0])
            nc.sync.dma_start(out=d_kt, in_=kt_sb[0])
            nc.sync.dma_start(out=d_vp, in_=vp_sb[0])

        # ---- phase 2: attention ----
        mpool = ctx.enter_context(tc.tile_pool(name="mask", bufs=2))
        p_pool = ctx.enter_context(tc.tile_pool(name="pexp", bufs=6))
        pm_pool = ctx.enter_context(tc.tile_pool(name="pmask", bufs=6))
        epi = ctx.enter_context(tc.tile_pool(name="epi", bufs=8))
        ot_pool = ctx.enter_context(tc.tile_pool(name="otsb", bufs=2))
        outsb_pool = ctx.enter_context(tc.tile_pool(name="outsb", bufs=4))

        def make_epilogue(qc, q0, otiles):
            def epi_fn():
                # all divisions first (frees O psum slots), then transposes +
                # projection draw psum from the freed "ot" slots so the next
                # q-chunk's QK/exp stream keeps the ring pool.
                otsb = ot_pool.tile([P, CH_T, QCHUNK], BF16, tag="otsb",
                                    name=f"otsb{qc}")
                osbs = []
                for s in range(QSUBS):
                    otv = otiles[s].rearrange("p (h e) -> p h e", h=HL)
                    zrec = epi.tile([P, HL], F32, tag="zr", name=f"zr{qc}_{s}")
                    nc.vector.reciprocal(zrec, otv[:, :, D])
                    osb = epi.tile([P, HL, D], BF16, tag="osb",
                                   name=f"osb{qc}_{s}")
                    zb = bass.AP(
                        tensor=zrec.tensor, offset=zrec.offset,
                        ap=[*zrec.ap, [0, D]])
                    nc.vector.tensor_mul(osb, otv[:, :, 0:D], zb)
                    osbs.append(osb)
                for s in range(QSUBS):
                    osf = osbs[s].rearrange("p h d -> p (h d)")
                    otp = o_pool.tile([P, CH_T, P], BF16, tag="ot",
                                      name=f"otp{qc}_{s}")
                    for ct in range(CH_T):
                        nc.tensor.transpose(
                            otp[:, ct, :], osf[:, ct * P:(ct + 1) * P], idn)
                    nc.vector.tensor_copy(otsb[:, :, s * P:(s + 1) * P], otp)
                    if debug and qc == 0 and s == 0:
                        nc.sync.dma_start(out=d_osb, in_=osbs[s])
                if debug and qc == 0:
                    nc.sync.dma_start(out=d_otsb, in_=otsb)
                for s in range(QSUBS):
                    ppss = []
                    for cf, (c0, c1) in enumerate(((0, CH), (CH, C))):
                        pps = o_pool.tile([P, CH], F32, tag="ot",
                                          name=f"pps{qc}_{s}_{cf}")
                        for ct in range(CH_T):
                            nc.tensor.matmul(
                                pps,
                                otsb[:, ct, s * P:(s + 1) * P],
                                wp_sb[ct][:, c0:c1],
                                start=(ct == 0), stop=(ct == CH_T - 1))
                        ppss.append(pps)
                    ob = outsb_pool.tile([P, C], F32, tag="ob",
                                         name=f"ob{qc}_{s}")
                    nc.vector.tensor_copy(ob[:, 0:CH], ppss[0])
                    nc.vector.tensor_copy(ob[:, CH:C], ppss[1])
                    nc.sync.dma_start(
                        out=out[q0 + s * P:q0 + (s + 1) * P, :], in_=ob)
            return epi_fn

        pending_epi = None
        for qc in range(NQC):
            q0 = qc * QCHUNK
            mk = mpool.tile([P, NKT, QCHUNK], BF16, tag="mk")
            for j in range(NKT):
                nc.sync.dma_start(
                    out=mk[:, j, :],
                    in_=negmt[j * P:(j + 1) * P, q0:q0 + QCHUNK])

            otiles = [o_pool.tile([P, HL * E], F32, tag="ot",
                                  name=f"otile{qc}_{s_}")
                      for s_ in range(QSUBS)]

            for h in range(HL):
                kth = kt_sb[h]
                qth = qt_sb[h]
                for ktp in range(NKT // 2):
                    ring = ring_pool.tile([P, 2, QCHUNK], F32, tag="ring")
                    for u in range(2):
                        kti = 2 * ktp + u
                        nc.tensor.matmul(
                            ring[:, u, :],
                            kth[:, kti * P:(kti + 1) * P],
                            qth[:, q0:q0 + QCHUNK],
                            start=True, stop=True)
                    pexp = p_pool.tile([P, 2, QCHUNK], BF16, tag="pe")
                    nc.scalar.activation(
                        pexp, ring, mybir.ActivationFunctionType.Exp)
                    pm = pm_pool.tile([P, 2, QCHUNK], BF16, tag="pm")
                    nc.vector.tensor_mul(
                        pm, pexp, mk[:, 2 * ktp:2 * ktp + 2, :])
                    if debug and qc == 0 and h == 0 and ktp == 0:
                        nc.sync.dma_start(out=d_pm, in_=pm)
                    for u in range(2):
                        kti = 2 * ktp + u
                        for s in range(QSUBS):
                            nc.tensor.matmul(
                                otiles[s][:, h * E:(h + 1) * E],
                                pm[:, u, s * P:(s + 1) * P],
                                vp_sb[kti][:, h, :],
                                start=(kti == 0), stop=(kti == NKT - 1))
                if h == 0 and pending_epi is not None:
                    pending_epi()
                    pending_epi = None
            pending_epi = make_epilogue(qc, q0, otiles)
        pending_epi()

    nc.compile()
    return nc


_CACHE = {}


def _get_nc():
    debug = os.environ.get("KERNEL_DEBUG", "0") == "1"
    key = ("nc", debug)
    if key not in _CACHE:
        _CACHE[key] = build_kernel(debug=debug)
    return _CACHE[key]


def kernel(x, mask, Wq, Wkv, Wp, bp):
    x = np.asarray(x, np.float32)
    mask = np.asarray(mask, np.float32)
    Wq = np.asarray(Wq, np.float32)
    Wkv = np.asarray(Wkv, np.float32)
    Wp = np.asarray(Wp, np.float32)
    bp = np.asarray(bp, np.float32)

    nc = _get_nc()
    in_maps = []
    for core in range(NCORES):
        b, hg = divmod(core, HGROUPS)
        rows = slice(hg * CH, (hg + 1) * CH)
        in_maps.append({
            "xT": np.ascontiguousarray(x[b].T.astype(ml_dtypes.bfloat16)),
            "wqt": np.ascontiguousarray(((Wq[rows, :] * SCALE).T).astype(ml_dtypes.bfloat16)),
            "wkt": np.ascontiguousarray(Wkv[rows, :].T.astype(ml_dtypes.bfloat16)),
            "wvt": np.ascontiguousarray(Wkv.T[:, C + hg * CH:C + (hg + 1) * CH].astype(ml_dtypes.bfloat16)),
            "wpt": np.ascontiguousarray(Wp[:, rows].T.astype(ml_dtypes.bfloat16)),
            "negmt": np.ascontiguousarray(
                (1.0 - mask[b].T).astype(ml_dtypes.bfloat16)),
        })

    trace = os.environ.get("KERNEL_TRACE", "0") == "1"
    res = run_bass_kernel_spmd(nc, in_maps, core_ids=list(range(NCORES)),
                               trace=trace)
    kernel.last_results = res

    outs = [res.results[i]["out"] for i in range(NCORES)]
    full = np.empty((B, N, C), np.float32)
    for b in range(B):
        full[b] = outs[2 * b] + outs[2 * b + 1] + bp[None, :]
    return full
